# revision 5
# baseline (speedup 1.0000x reference)
"""Trainium2 Bass kernel for nn_NeuralODE: 196 Tsit5 steps of a 3->64->64->3
tanh MLP vector field over batch 32768, data-parallel across 8 NeuronCores.

Mathematical reformulation (keeps the PE at full 128x128 utilization):
  State per batch row is zb := y @ W1 + b1  (64-dim) instead of y (3-dim).
  With G := W3 @ W1, g0 := b3 @ W1, each stage input in zb-space is
     zin_i = zb + sum_{j<i} (h*A_ij) * (h2_j @ G) + (h*sumA_i) * g0
  and the step update is
     zb' = zb + sum_i (h*B_i) * (h2_i @ G) + (h*sumB) * g0.
  y is only needed at save points: y = (zb - b1) @ pinv(W1).
  All constant-vector terms are folded into per-stage ACT bias operands.

Layout per core: batch shard 4096 rows = 2 waves x 2048 rows; each wave is
packed [128 partitions = 64 feats x 2 batch-halves, 1024 free].  All matmuls
use block-diagonal duplicated weights so K=128 (full PE array).  Matmul inputs
use float32r (fp32 with 11-bit mantissa, full-rate on the PE); state stays
fp32; the save matmul runs in exact fp32.
"""
import numpy as np

import concourse.bacc as bacc
import concourse.bass as bass
import concourse.mybir as mybir
from concourse.bass import ds
from concourse.tile import TileContext
from concourse.bass_utils import run_bass_kernel_spmd

F32 = mybir.dt.float32
F32R = mybir.dt.float32r
TANH = mybir.ActivationFunctionType.Tanh
IDENT = mybir.ActivationFunctionType.Identity

N_CORES = 8
T, B, D, W = 50, 32768, 3, 64
SUB = 4
N_INT = T - 1          # 49 save intervals
WAVES = 2
FREE = B // N_CORES // WAVES // 2   # 1024: packed free dim per wave
HALF = FREE                         # batch rows per half

# Tsit5 tableau (matches reference.py)
_A = np.zeros((7, 7))
_A[2, 1] = 0.161
_A[3, 1], _A[3, 2] = -0.008480655492356989, 0.335480655492357
_A[4, 1], _A[4, 2], _A[4, 3] = 2.8971530571054935, -6.359448489975075, 4.3622954328695815
_A[5, 1], _A[5, 2], _A[5, 3], _A[5, 4] = (
    5.325864828439257, -11.748883564062828, 7.4955393428898365, -0.09249506636175525)
_A[6, 1], _A[6, 2], _A[6, 3], _A[6, 4], _A[6, 5] = (
    5.86145544294642, -12.92096931784711, 8.159367898576159,
    -0.071584973281401, -0.028269050394068383)
_B = np.array([0.0, 0.09646076681806523, 0.01, 0.4798896504144996,
               1.379008574103742, -3.290069515436081, 2.324710524099774])

_GS_PAIRS = [(i, j) for i in range(2, 7) for j in range(1, i)]   # 15
_GS_IDX = {p: k for k, p in enumerate(_GS_PAIRS)}
W2_IDX = 21   # wts slot of W2 block
GB_IDX = 15   # wts slots 15..20 are Gb_1..Gb_6

LAST_EXEC_NS = None


def _round_fp32r(x: np.ndarray) -> np.ndarray:
    """Round fp32 array to the fp32r grid (11-bit mantissa, RNE-ish)."""
    u = np.ascontiguousarray(np.asarray(x, dtype=np.float32)).view(np.uint32)
    r = (u + np.uint32(0x7FF) + ((u >> np.uint32(12)) & np.uint32(1))) & np.uint32(0xFFFFF000)
    return r.view(np.float32)


def _blk(m64: np.ndarray) -> np.ndarray:
    """Duplicate a [64,64] matrix into a block-diagonal [128,128]."""
    z = np.zeros((128, 128), dtype=np.float64)
    z[0:64, 0:64] = m64
    z[64:128, 64:128] = m64
    return z


def build(n_intervals: int = N_INT, body_reps: int = 1, loop_mult: int = 1,
          static_save: bool = False):
    nc = bacc.Bacc(None, target_bir_lowering=False)

    zb0_d = nc.dram_tensor("zb0", [WAVES, 128, FREE], F32, kind="ExternalInput")
    wts_d = nc.dram_tensor("wts", [22, 128, 128], F32R, kind="ExternalInput")
    pblk_d = nc.dram_tensor("pblk", [128, 8], F32, kind="ExternalInput")
    bia_d = nc.dram_tensor("biases", [128, 28], F32, kind="ExternalInput")
    ys_d = nc.dram_tensor("ys", [n_intervals, 6 * WAVES, FREE], F32,
                          kind="ExternalOutput")

    with TileContext(nc) as tc:
        with tc.tile_pool(name="wpool", bufs=1) as wpool, \
             tc.tile_pool(name="spool", bufs=1) as spool, \
             tc.tile_pool(name="h1pool", bufs=3) as h1pool, \
             tc.tile_pool(name="zspool", bufs=3) as zspool, \
             tc.tile_pool(name="yspool", bufs=2) as yspool, \
             tc.tile_pool(name="psz", bufs=1, space="PSUM") as pszpool, \
             tc.tile_pool(name="psw", bufs=1, space="PSUM") as pswpool:

            wt = []
            for k in range(22):
                t = wpool.tile([128, 128], F32R, name=f"wt{k}")
                nc.sync.dma_start(out=t[:, :], in_=wts_d[k, :, :])
                wt.append(t)
            pb = wpool.tile([128, 8], F32, name="pb")
            nc.sync.dma_start(out=pb[:, :], in_=pblk_d[:, :])
            bia = wpool.tile([128, 28], F32, name="bia")
            nc.sync.dma_start(out=bia[:, :], in_=bia_d[:, :])

            zbt = []
            for w in range(WAVES):
                pair = []
                for s in range(2):
                    t = spool.tile([128, FREE], F32, name=f"zbt{w}_{s}")
                    pair.append(t)
                nc.sync.dma_start(out=pair[0][:, :], in_=zb0_d[w, :, :])
                zbt.append(pair)
            h2 = [[spool.tile([128, FREE], F32R, name=f"h2_{w}_{i}")
                   for i in range(6)] for w in range(WAVES)]

            # warm up the ACT table set (tanh) outside the loop
            wu = wpool.tile([128, 1], F32R, name="wu")
            nc.scalar.activation(wu[:, :], bia[:, 27:28], TANH)

            def mm_w2(w, h1t, h2dst, bias_col):
                wp = pswpool.tile([128, FREE], F32, name="wp", tag=f"w{w}")
                for c in range(2):
                    cs = slice(c * 512, (c + 1) * 512)
                    nc.tensor.matmul(wp[:, cs], wt[W2_IDX][:, :], h1t[:, cs],
                                     start=True, stop=True)
                nc.scalar.activation(h2dst[:, :], wp[:, :], TANH,
                                     bias=bias_col, scale=1.0)

            def emit_step(w, sig, zcur, znxt):
                b2c = bia[:, 24:25]
                # stage 1: zin = zcur (+ bias)
                h1 = h1pool.tile([128, FREE], F32R, name="h1", tag="h1")
                nc.scalar.activation(h1[:, :], zcur[:, :], TANH,
                                     bias=bia[:, sig * 6:sig * 6 + 1], scale=1.0)
                mm_w2(w, h1, h2[w][0], b2c)
                for i in range(2, 7):
                    zp = pszpool.tile([128, FREE], F32, name="zp", tag=f"z{w}")
                    for j in range(1, i):
                        g = wt[_GS_IDX[(i, j)]]
                        for c in range(2):
                            cs = slice(c * 512, (c + 1) * 512)
                            nc.tensor.matmul(zp[:, cs], g[:, :], h2[w][j - 1][:, cs],
                                             start=(j == 1), stop=(j == i - 1),
                                             skip_group_check=True)
                    zs = zspool.tile([128, FREE], F32, name="zs", tag="zs")
                    nc.vector.tensor_add(out=zs[:, :], in0=zp[:, :], in1=zcur[:, :])
                    h1 = h1pool.tile([128, FREE], F32R, name="h1", tag="h1")
                    nc.scalar.activation(h1[:, :], zs[:, :], TANH,
                                         bias=bia[:, sig * 6 + i - 1:sig * 6 + i],
                                         scale=1.0)
                    mm_w2(w, h1, h2[w][i - 1], b2c)
                # state update: znxt = zcur + sum_i Gb_i @ h2_i
                dp = pszpool.tile([128, FREE], F32, name="dp", tag=f"z{w}")
                for i in range(1, 7):
                    for c in range(2):
                        cs = slice(c * 512, (c + 1) * 512)
                        nc.tensor.matmul(dp[:, cs], wt[GB_IDX + i - 1][:, :],
                                         h2[w][i - 1][:, cs],
                                         start=(i == 1), stop=(i == 6),
                                         skip_group_check=True)
                nc.vector.tensor_add(out=znxt[:, :], in0=dp[:, :], in1=zcur[:, :])
                if sig == SUB - 1:
                    # re-add the 4 deferred (h*sumB)*g0 constants
                    nc.vector.tensor_scalar_add(znxt[:, :], znxt[:, :], bia[:, 25:26])

            def emit_save(w, iv, z):
                yp = pszpool.tile([6, FREE], F32, name="yp", tag=f"z{w}")
                for c in range(2):
                    cs = slice(c * 512, (c + 1) * 512)
                    nc.tensor.matmul(yp[:, cs], pb[:, 0:6], z[:, cs],
                                     start=True, stop=True)
                ysb = yspool.tile([6, FREE], F32, name="ysb", tag="ysb")
                nc.scalar.activation(ysb[:, :], yp[:, :], IDENT,
                                     bias=bia[0:6, 26:27], scale=1.0)
                if static_save:
                    nc.sync.dma_start(out=ys_d[0, 6 * w:6 * w + 6, :],
                                      in_=ysb[:, :])
                else:
                    nc.sync.dma_start(out=ys_d[ds(iv, 1), 6 * w:6 * w + 6, :],
                                      in_=ysb[:, :])

            with tc.For_i(0, n_intervals * loop_mult, 1,
                          hint_engines=(mybir.EngineType.PE,)) as iv:
                for _rep in range(body_reps):
                    for sig in range(SUB):
                        cur = sig % 2
                        for w in range(WAVES):
                            emit_step(w, sig, zbt[w][cur], zbt[w][1 - cur])
                for w in range(WAVES):
                    emit_save(w, iv, zbt[w][0])

    nc.finalize()
    return nc


def build_timing_double(n_intervals: int = N_INT):
    """Timing-only variant: two interval bodies per save (wrong results)."""
    return build(n_intervals, body_reps=2)


_nc_cache = {}


def _get_nc(n_intervals: int):
    if n_intervals not in _nc_cache:
        _nc_cache[n_intervals] = build(n_intervals)
    return _nc_cache[n_intervals]


def prep_inputs(ts, y0, W1, b1, W2, b2, W3, b3):
    """Host-side precompute (float64) -> per-core input maps."""
    ts64 = np.asarray(ts, dtype=np.float64)
    h = (ts64[1] - ts64[0]) / SUB
    W1_, b1_, W2_, b2_, W3_, b3_ = [np.asarray(a, dtype=np.float64)
                                    for a in (W1, b1, W2, b2, W3, b3)]
    y0_ = np.asarray(y0, dtype=np.float64)

    G = W3_ @ W1_                       # [64, 64]
    g0 = b3_ @ W1_                      # [64]
    P = np.linalg.pinv(W1_)             # [64, 3]
    sumB = _B.sum()
    g0pk = np.concatenate([g0, g0])     # [128]

    wts = np.zeros((22, 128, 128), dtype=np.float64)
    for (i, j), k in _GS_IDX.items():
        wts[k] = _blk(h * _A[i, j] * G)
    for i in range(1, 7):
        wts[GB_IDX + i - 1] = _blk(h * _B[i] * G)
    wts[W2_IDX] = _blk(W2_)
    wts = _round_fp32r(wts.astype(np.float32))

    pblk = np.zeros((128, 8), dtype=np.float64)
    for hh in range(2):
        pblk[hh * 64:(hh + 1) * 64, hh * 3:(hh + 1) * 3] = P
    pblk = pblk.astype(np.float32)

    bia = np.zeros((128, 28), dtype=np.float64)
    for sig in range(SUB):
        for i in range(1, 7):
            sumA = _A[i, 1:i].sum()
            bia[:, sig * 6 + i - 1] = (h * sumA + sig * h * sumB) * g0pk
    bia[:, 24] = np.concatenate([b2_, b2_])
    bia[:, 25] = SUB * h * sumB * g0pk
    yb = -(b1_ @ P)                     # [3]
    for hh in range(2):
        bia[hh * 3:hh * 3 + 3, 26] = yb
    bia = bia.astype(np.float32)

    zb0 = (y0_ @ W1_ + b1_).astype(np.float32)        # [B, 64]
    # pack: [core, wave, half, n, f] -> [core, wave, half*64+f, n]
    zb0 = zb0.reshape(N_CORES, WAVES, 2, HALF, W).transpose(0, 1, 2, 4, 3) \
             .reshape(N_CORES, WAVES, 128, FREE)
    zb0 = np.ascontiguousarray(zb0)

    in_maps = []
    for c in range(N_CORES):
        in_maps.append({
            "zb0": np.ascontiguousarray(zb0[c]),
            "wts": wts,
            "pblk": pblk,
            "biases": bia,
        })
    return in_maps


def assemble(results, y0, n_intervals: int = N_INT):
    """Per-core ys [n_int, 12, 1024] -> full [n_int+1, B, 3]."""
    y0 = np.asarray(y0, dtype=np.float32)
    ys = np.empty((n_intervals + 1, B, 3), dtype=np.float32)
    ys[0] = y0
    shard = B // N_CORES
    for c in range(N_CORES):
        o = np.asarray(results[c]["ys"])
        # [t, w, h, d, n] -> [t, w, h, n, d]
        o = o.reshape(n_intervals, WAVES, 2, 3, FREE).transpose(0, 1, 2, 4, 3) \
             .reshape(n_intervals, shard, 3)
        ys[1:, c * shard:(c + 1) * shard, :] = o
    return ys


def kernel(ts, y0, W1, b1, W2, b2, W3, b3):
    global LAST_EXEC_NS
    in_maps = prep_inputs(ts, y0, W1, b1, W2, b2, W3, b3)
    nc = _get_nc(N_INT)
    res = run_bass_kernel_spmd(nc, in_maps, list(range(N_CORES)))
    LAST_EXEC_NS = res.exec_time_ns
    return assemble(res.results, y0, N_INT)


if __name__ == "__main__":
    # smoke test with tiny interval count against a numpy reference
    rng = np.random.default_rng(0)
    ts = np.linspace(0, 1, T, dtype=np.float32)
    y0 = rng.standard_normal((B, D)).astype(np.float32)
    W1 = (rng.standard_normal((D, W)) / np.sqrt(D)).astype(np.float32)
    W2 = (rng.standard_normal((W, W)) / np.sqrt(W)).astype(np.float32)
    W3 = (rng.standard_normal((W, D)) / np.sqrt(W)).astype(np.float32)
    b1 = np.zeros(W, np.float32)
    b2 = np.zeros(W, np.float32)
    b3 = np.zeros(D, np.float32)

    n_int = 2
    in_maps = prep_inputs(ts, y0, W1, b1, W2, b2, W3, b3)
    nc = build(n_int)
    res = run_bass_kernel_spmd(nc, in_maps, list(range(N_CORES)))
    ys = assemble(res.results, y0, n_int)

    # numpy reference (float64) for the first n_int*SUB steps
    def vf(y):
        h1 = np.tanh(y @ W1.astype(np.float64) + b1)
        hh = np.tanh(h1 @ W2.astype(np.float64) + b2)
        return hh @ W3.astype(np.float64) + b3

    h = float(ts[1] - ts[0]) / SUB
    y = y0.astype(np.float64)
    outs = [y0.astype(np.float64)]
    for t in range(n_int * SUB):
        k1 = vf(y)
        k2 = vf(y + h * (_A[2, 1] * k1))
        k3 = vf(y + h * (_A[3, 1] * k1 + _A[3, 2] * k2))
        k4 = vf(y + h * (_A[4, 1] * k1 + _A[4, 2] * k2 + _A[4, 3] * k3))
        k5 = vf(y + h * (_A[5, 1] * k1 + _A[5, 2] * k2 + _A[5, 3] * k3 + _A[5, 4] * k4))
        k6 = vf(y + h * (_A[6, 1] * k1 + _A[6, 2] * k2 + _A[6, 3] * k3
                         + _A[6, 4] * k4 + _A[6, 5] * k5))
        y = y + h * (_B[1] * k1 + _B[2] * k2 + _B[3] * k3 + _B[4] * k4
                     + _B[5] * k5 + _B[6] * k6)
        if (t + 1) % SUB == 0:
            outs.append(y.copy())
    ref = np.stack(outs)
    err = np.abs(ys - ref).max()
    scale = np.abs(ref).max()
    print(f"smoke n_int={n_int}: maxabs={err:.3e} rel={err/scale:.3e} scale={scale:.3f}")
